# revision 1
# baseline (speedup 1.0000x reference)
"""HMM log-likelihood (log-domain forward algorithm) on 8 Trainium2 cores.

Strategy: scaled linear-domain forward algorithm with warmup-halo sequence
parallelism.  The filtering distribution of an HMM forgets its initial
condition geometrically fast, so N=1e6 timesteps are split into 3840
independent chains (480/core); each chain starts from a uniform state W=20
steps before its owned region of L=260 steps.  Per core, chains are batched
4-wide across the 128 SBUF partitions (block-diagonal T^T weights on the PE)
with the chain-block index in the matmul free dimension, so each timestep is
one bf16 matmul (T @ S into PSUM) plus one vector multiply by the emission
probabilities.

Normalization is free: a constant per-step drift delta = E[log c] is folded
into the exp bias, making log|S| a zero-drift random walk (~26 bits 4.5
sigma over a 280-step chain — far inside f32 range), so the kernel needs no
per-chain rescaling.  The bf16 quantization of T factors exactly as
D_r @ T_hat with T_hat row-stochastic; -log(r) is folded into the same exp
bias.  Each chain's contribution is log(sum(S_final)) - log(sum(S_at_W)) +
delta*L, assembled on the host, which also runs exact f64 scans for the
prefix [0, W) and the short tail.
"""

import sys

for p in ("/opt/trn_rl_repo", "/root/.axon_site", "/root/.axon_site/_ro/trn_rl_repo",
          "/root/.axon_site/_ro/pypackages"):
    if p not in sys.path:
        sys.path.insert(0, p)

import numpy as np

K = 32
N = 1_000_000
NCORES = 8
W = 20            # warmup (halo) steps per chain
L = 260           # owned steps per chain
CC = 480          # chains per core
SPAN = W + L      # 280 sequential steps
SBLK = 140        # timesteps per load window
NWIN = SPAN // SBLK
NB = CC // 4      # 120 four-chain blocks
G = 2             # interleaved compute groups
F = NB // G       # 60 blocks (matmul free dim) per group
NSL = CC * L + W  # per-core input slice columns
COVERED = W + NCORES * CC * L

_cache = {}


def _build():
    import concourse.bass as bass
    import concourse.bacc as bacc
    import concourse.mybir as mybir
    import concourse.tile as tile
    from contextlib import ExitStack

    f32 = mybir.dt.float32
    bf16 = mybir.dt.bfloat16
    AF = mybir.ActivationFunctionType

    nc = bacc.Bacc("TRN2", target_bir_lowering=False, debug=False,
                   num_devices=NCORES)
    x = nc.dram_tensor("x", [K, NSL], f32, kind="ExternalInput")
    wmat = nc.dram_tensor("wmat", [128, 128], bf16, kind="ExternalInput")
    ebias = nc.dram_tensor("ebias", [128, 1], f32, kind="ExternalInput")
    snap_out = nc.dram_tensor("snap_out", [128, NB], bf16, kind="ExternalOutput")
    fin_out = nc.dram_tensor("fin_out", [128, NB], bf16, kind="ExternalOutput")

    with tile.TileContext(nc) as tc:
        with ExitStack() as ctx:
            cpool = ctx.enter_context(tc.tile_pool(name="const", bufs=1))
            rpool = ctx.enter_context(tc.tile_pool(name="rp", bufs=NWIN))
            pspool = ctx.enter_context(
                tc.tile_pool(name="ps", bufs=2, space=bass.MemorySpace.PSUM))

            w_t = cpool.tile([128, 128], bf16, tag="w")
            nc.sync.dma_start(w_t[:], wmat[:])
            eb_t = cpool.tile([128, 1], f32, tag="eb")
            nc.sync.dma_start(eb_t[:], ebias[:])

            spool = ctx.enter_context(tc.tile_pool(name="sp", bufs=2))
            S, SN = [], []
            for g in range(G):
                st = spool.tile([128, F], bf16, tag=f"S{g}", name=f"st{g}")
                nc.vector.memset(st[:], 1.0)
                sn = cpool.tile([128, F], bf16, tag=f"N{g}")
                S.append(st)
                SN.append(sn)

            # Load + exp windows.  R[g][w] layout: [128, F, SBLK], partition
            # p = 32*q + k holds chain (g*F + cb)*4 + q, state k.
            R = [[None] * NWIN for _ in range(G)]
            NCHUNK = 4
            CH = F // NCHUNK
            # interleave DMA chunks and exp chunks across groups so both
            # chains become runnable at the same (early) time
            for w in range(NWIN):
                for g in range(G):
                    rt = rpool.tile([128, F, SBLK], f32, tag=f"R{g}",
                                    name=f"rt{g}_{w}")
                    R[g][w] = rt
                for ch in range(NCHUNK):
                    for g in range(G):
                        rt = R[g][w]
                        cb0 = ch * CH
                        for q in range(4):
                            off = ((g * F + cb0) * 4 + q) * L + w * SBLK
                            src = bass.AP(x, off,
                                          [[NSL, 32], [4 * L, CH], [1, SBLK]])
                            nc.sync.dma_start(
                                rt[32 * q:32 * q + 32, cb0:cb0 + CH, :], src)
                # exp in place, chunked along s so compute starts early
                EC = 7
                for ec in range(EC):
                    for g in range(G):
                        rt = R[g][w]
                        s0 = ec * (SBLK // EC)
                        nc.scalar.activation(
                            rt[:, :, s0:s0 + SBLK // EC],
                            rt[:, :, s0:s0 + SBLK // EC], AF.Exp,
                            bias=eb_t[:])

            for s in range(SPAN):
                w, si = divmod(s, SBLK)
                for g in range(G):
                    ps = pspool.tile([128, F], f32, tag=f"mm{g}")
                    nc.tensor.matmul(ps[:], w_t[:], S[g][:], start=True, stop=True)
                    # ping-pong the state tile so the multiply never WARs
                    # against this step's matmul read
                    sn_new = spool.tile([128, F], bf16, tag=f"S{g}",
                                        name=f"st{g}_{s}")
                    nc.vector.tensor_mul(sn_new[:], ps[:], R[g][w][:, :, si])
                    S[g] = sn_new
                    if s == W - 1:
                        nc.vector.tensor_copy(SN[g][:], S[g][:])

            for g in range(G):
                nc.sync.dma_start(snap_out[:, g * F:(g + 1) * F], SN[g][:])
                nc.sync.dma_start(fin_out[:, g * F:(g + 1) * F], S[g][:])

    nc.compile()
    return nc


def _get_nc():
    if "nc" not in _cache:
        _cache["nc"] = _build()
    return _cache["nc"]


def _log_softmax64(v, axis):
    v = v.astype(np.float64)
    m = v.max(axis=axis, keepdims=True)
    e = np.exp(v - m)
    return v - m - np.log(e.sum(axis=axis, keepdims=True))


def _estimate_delta(log_pdf, T64):
    # E[log c] from a vectorized short scan: 64 parallel probes, 56 steps,
    # burn-in 16 (mixing time is ~10 steps).
    NCH, NST, BURN = 64, 56, 16
    cols = np.arange(NCH) * 997 + 1
    a = np.full((K, NCH), 1.0 / K)
    samples = []
    for s in range(NST):
        p = np.exp(log_pdf[:, cols + s].astype(np.float64))
        a = p * (T64 @ a)
        c = a.sum(axis=0)
        a /= c
        if s >= BURN:
            samples.append(np.log(c))
    return float(np.mean(samples))


def _make_in_maps(log_pdf, T64):
    from ml_dtypes import bfloat16

    T32 = T64.astype(np.float32)
    Tbf = T32.astype(bfloat16)
    delta = _estimate_delta(log_pdf, T64)
    # bf16-quantized T is exactly D_r @ T_hat with T_hat row-stochastic and
    # r the bf16 row sums; fold -log(r) and the drift -delta into the exp.
    r = Tbf.astype(np.float64).sum(axis=1)
    eb = np.zeros((128, 1), dtype=np.float32)
    for q in range(4):
        eb[32 * q:32 * q + 32, 0] = (-np.log(r) - delta).astype(np.float32)
    wm = np.zeros((128, 128), dtype=bfloat16)
    for q in range(4):
        wm[32 * q:32 * q + 32, 32 * q:32 * q + 32] = Tbf.T
    in_maps = []
    for k in range(NCORES):
        c0 = k * CC * L
        in_maps.append({
            "x": np.ascontiguousarray(log_pdf[:, c0:c0 + NSL]),
            "wmat": wm,
            "ebias": eb,
        })

    return in_maps, delta


def kernel(log_pdf: np.ndarray, pi: np.ndarray, T: np.ndarray) -> np.ndarray:
    from concourse.bass_utils import run_bass_kernel_spmd

    log_pdf = np.ascontiguousarray(log_pdf, dtype=np.float32)
    log_pi64 = _log_softmax64(pi, 0)
    log_T64 = _log_softmax64(T, 1)
    T64 = np.exp(log_T64)                     # row-stochastic [K, K] f64

    in_maps, delta = _make_in_maps(log_pdf, T64)
    nc = _get_nc()
    res = run_bass_kernel_spmd(nc, in_maps, list(range(NCORES))).results

    # ---- host combine (f64) ----
    LP = log_pdf
    # exact prefix [0, W)
    a = np.exp(log_pi64 + LP[:, 0].astype(np.float64))
    c = a.sum()
    total = np.log(c)
    a /= c
    for t in range(1, W):
        a = np.exp(LP[:, t].astype(np.float64)) * (T64 @ a)
        c = a.sum()
        total += np.log(c)
        a /= c

    # per-chain contributions: log(sum fin) - log(sum snap) + delta*L
    for k in range(NCORES):
        snap = res[k]["snap_out"].astype(np.float64)   # [128, NB]
        fin = res[k]["fin_out"].astype(np.float64)
        for q in range(4):
            ssum = snap[32 * q:32 * q + 32, :].sum(axis=0)
            fsum = fin[32 * q:32 * q + 32, :].sum(axis=0)
            total += (np.log(fsum) - np.log(ssum)).sum() + delta * L * NB

    # exact tail [COVERED, N) from the last chain's final state
    k, g, cb, q = NCORES - 1, G - 1, F - 1, 3
    fv = res[k]["fin_out"][32 * q:32 * q + 32, g * F + cb].astype(np.float64)
    a = fv / fv.sum()
    for t in range(COVERED, N):
        a = np.exp(LP[:, t].astype(np.float64)) * (T64 @ a)
        c = a.sum()
        total += np.log(c)
        a /= c

    return np.float32(total)



# revision 3
# speedup vs baseline: 2.0396x; 2.0396x over previous
"""HMM log-likelihood (log-domain forward algorithm) on 8 Trainium2 cores.

Strategy: scaled linear-domain forward algorithm with warmup-halo sequence
parallelism, step-major host layout.  N=1e6 timesteps split into 8*CC
independent chains (CC/core); each chain starts from a uniform state W steps
before its owned region of L steps.  The host rearranges the per-core input
slice into X[s, partition, chain] (step-major), so the device DMA is large
contiguous packets that arrive in exactly the order the scan consumes them:
the scan overlaps the HBM stream instead of waiting behind it.

Per core, chains are batched 4-wide across the 128 SBUF partitions
(block-diagonal T^T weights on the PE) with the chain index in the matmul
free dimension; G=2 interleaved groups hide the matmul->multiply latency.
Each timestep is one bf16 matmul per group (T @ S into PSUM) plus one vector
multiply by the emission probabilities, split in half across the Vector and
GpSimd engines.  Exp (with folded bias) runs on the Scalar engine, windowed
behind the DMA.

Normalization is free: a constant per-step drift delta = E[log c] is folded
into the exp bias, making log|S| a zero-drift random walk, so no per-chain
rescaling is needed.  The bf16 quantization of T factors exactly as
D_r @ T_hat with T_hat row-stochastic; -log(r) is folded into the same exp
bias.  Each chain's contribution is log(sum(S_final)) - log(sum(S_at_W)) +
delta*L, assembled on the host, which also runs exact f64 scans for the
prefix [0, W) and the short tail.
"""

import sys

for p in ("/opt/trn_rl_repo", "/root/.axon_site", "/root/.axon_site/_ro/trn_rl_repo",
          "/root/.axon_site/_ro/pypackages"):
    if p not in sys.path:
        sys.path.insert(0, p)

import numpy as np

K = 32
N = 1_000_000
NCORES = 8
W = 8             # warmup (halo) steps per chain
L = 40            # owned steps per chain
CC = 3120         # chains per core
SPAN = W + L      # 48 sequential steps
CQ = CC // 4      # 780 chains per partition group
G = 2             # interleaved compute groups
F = CQ // G       # 390 chains (matmul free dim) per group
FH = F // 2       # 195-wide halves for the DVE/GpSimd multiply split
SB = 6            # timesteps per DMA/exp window
NW = SPAN // SB   # 8 windows
COVERED = W + NCORES * CC * L

_cache = {}


def _build():
    import concourse.bass as bass
    import concourse.bacc as bacc
    import concourse.mybir as mybir
    import concourse.tile as tile
    from contextlib import ExitStack

    f32 = mybir.dt.float32
    bf16 = mybir.dt.bfloat16
    AF = mybir.ActivationFunctionType

    nc = bacc.Bacc("TRN2", target_bir_lowering=False, debug=False,
                   num_devices=NCORES)
    # step-major input: X[s, p, j] with p = 32*q + k, chain = q*CQ + j
    x = nc.dram_tensor("x", [SPAN * 128, CQ], f32, kind="ExternalInput")
    wmat = nc.dram_tensor("wmat", [128, 128], bf16, kind="ExternalInput")
    ebias = nc.dram_tensor("ebias", [128, 1], f32, kind="ExternalInput")
    snap_out = nc.dram_tensor("snap_out", [128, CQ], bf16, kind="ExternalOutput")
    fin_out = nc.dram_tensor("fin_out", [128, CQ], bf16, kind="ExternalOutput")

    with tile.TileContext(nc) as tc:
        with ExitStack() as ctx:
            cpool = ctx.enter_context(tc.tile_pool(name="const", bufs=1))
            rpool = ctx.enter_context(tc.tile_pool(name="rp", bufs=1))
            spool = ctx.enter_context(tc.tile_pool(name="sp", bufs=2))
            pspool = ctx.enter_context(
                tc.tile_pool(name="ps", bufs=2, space=bass.MemorySpace.PSUM))

            w_t = cpool.tile([128, 128], bf16, tag="w")
            nc.sync.dma_start(w_t[:], wmat[:])
            eb_t = cpool.tile([128, 1], f32, tag="eb")
            nc.sync.dma_start(eb_t[:], ebias[:])

            # all SPAN steps of emission data, step-major, exp'd in place
            rt = rpool.tile([128, SPAN, CQ], f32, tag="R")

            S, SN = [], []
            for g in range(G):
                st = spool.tile([128, F], bf16, tag=f"S{g}", name=f"st{g}")
                nc.vector.memset(st[:], 1.0)
                sn = cpool.tile([128, F], bf16, tag=f"N{g}")
                S.append(st)
                SN.append(sn)

            # window DMAs (4 per window, split by partition quarter) + exp
            for w in range(NW):
                s0 = w * SB
                for q in range(4):
                    src = bass.AP(x, (s0 * 128 + 32 * q) * CQ,
                                  [[CQ, 32], [128 * CQ, SB], [1, CQ]])
                    nc.sync.dma_start(rt[32 * q:32 * q + 32, s0:s0 + SB, :], src)
                nc.scalar.activation(rt[:, s0:s0 + SB, :],
                                     rt[:, s0:s0 + SB, :], AF.Exp,
                                     bias=eb_t[:])

            for s in range(SPAN):
                for g in range(G):
                    ps = pspool.tile([128, F], f32, tag=f"mm{g}")
                    nc.tensor.matmul(ps[:], w_t[:], S[g][:], start=True,
                                     stop=True)
                    sn_new = spool.tile([128, F], bf16, tag=f"S{g}",
                                        name=f"st{g}_{s}")
                    c0 = g * F
                    nc.vector.tensor_mul(sn_new[:], ps[:],
                                         rt[:, s, c0:c0 + F])
                    S[g] = sn_new
                    if s == W - 1:
                        nc.gpsimd.tensor_copy(SN[g][:], S[g][:])

            for g in range(G):
                nc.sync.dma_start(snap_out[:, g * F:(g + 1) * F], SN[g][:])
                nc.sync.dma_start(fin_out[:, g * F:(g + 1) * F], S[g][:])

    nc.compile()
    return nc


def _get_nc():
    if "nc" not in _cache:
        _cache["nc"] = _build()
    return _cache["nc"]


def _log_softmax64(v, axis):
    v = v.astype(np.float64)
    m = v.max(axis=axis, keepdims=True)
    e = np.exp(v - m)
    return v - m - np.log(e.sum(axis=axis, keepdims=True))


def _estimate_delta(log_pdf, T64):
    # E[log c] from a vectorized short scan: 64 parallel probes, 56 steps,
    # burn-in 16 (mixing time is ~10 steps).
    NCH, NST, BURN = 64, 56, 16
    cols = np.arange(NCH) * 997 + 1
    a = np.full((K, NCH), 1.0 / K)
    samples = []
    for s in range(NST):
        p = np.exp(log_pdf[:, cols + s].astype(np.float64))
        a = p * (T64 @ a)
        c = a.sum(axis=0)
        a /= c
        if s >= BURN:
            samples.append(np.log(c))
    return float(np.mean(samples))


def _make_in_maps(log_pdf, T64):
    from ml_dtypes import bfloat16

    T32 = T64.astype(np.float32)
    Tbf = T32.astype(bfloat16)
    delta = _estimate_delta(log_pdf, T64)
    # bf16-quantized T is exactly D_r @ T_hat with T_hat row-stochastic and
    # r the bf16 row sums; fold -log(r) and the drift -delta into the exp.
    r = Tbf.astype(np.float64).sum(axis=1)
    eb = np.zeros((128, 1), dtype=np.float32)
    for q in range(4):
        eb[32 * q:32 * q + 32, 0] = (-np.log(r) - delta).astype(np.float32)
    wm = np.zeros((128, 128), dtype=bfloat16)
    for q in range(4):
        wm[32 * q:32 * q + 32, 32 * q:32 * q + 32] = Tbf.T

    # step-major gather: X[s, 32q+k, j] = log_pdf[k, m*CC*L + (q*CQ+j)*L + s]
    col = np.arange(CC, dtype=np.int64) * L          # [CC]
    step = np.arange(SPAN, dtype=np.int64)           # [SPAN]
    idx0 = col[None, :] + step[:, None]              # [SPAN, CC]
    in_maps = []
    for m in range(NCORES):
        idx = m * CC * L + idx0
        xm = log_pdf[:, idx]                         # [K, SPAN, CC]
        # -> [SPAN, 4, K, CQ] -> [SPAN*128, CQ]
        xm = xm.reshape(K, SPAN, 4, CQ).transpose(1, 2, 0, 3)
        xm = np.ascontiguousarray(xm, dtype=np.float32).reshape(SPAN * 128, CQ)
        in_maps.append({"x": xm, "wmat": wm, "ebias": eb})

    return in_maps, delta


def kernel(log_pdf: np.ndarray, pi: np.ndarray, T: np.ndarray) -> np.ndarray:
    from concourse.bass_utils import run_bass_kernel_spmd

    log_pdf = np.ascontiguousarray(log_pdf, dtype=np.float32)
    log_pi64 = _log_softmax64(pi, 0)
    log_T64 = _log_softmax64(T, 1)
    T64 = np.exp(log_T64)                     # row-stochastic [K, K] f64

    in_maps, delta = _make_in_maps(log_pdf, T64)
    nc = _get_nc()
    res = run_bass_kernel_spmd(nc, in_maps, list(range(NCORES))).results

    # ---- host combine (f64) ----
    LP = log_pdf
    # exact prefix [0, W)
    a = np.exp(log_pi64 + LP[:, 0].astype(np.float64))
    c = a.sum()
    total = np.log(c)
    a /= c
    for t in range(1, W):
        a = np.exp(LP[:, t].astype(np.float64)) * (T64 @ a)
        c = a.sum()
        total += np.log(c)
        a /= c

    # per-chain contributions: log(sum fin) - log(sum snap) + delta*L
    for m in range(NCORES):
        snap = res[m]["snap_out"].astype(np.float64)   # [128, CQ]
        fin = res[m]["fin_out"].astype(np.float64)
        for q in range(4):
            ssum = snap[32 * q:32 * q + 32, :].sum(axis=0)
            fsum = fin[32 * q:32 * q + 32, :].sum(axis=0)
            total += (np.log(fsum) - np.log(ssum)).sum() + delta * L * CQ

    # exact tail [COVERED, N) from the last chain's final state
    # last chain = (core NCORES-1, q=3, j=CQ-1)
    fv = res[NCORES - 1]["fin_out"][96:128, CQ - 1].astype(np.float64)
    a = fv / fv.sum()
    for t in range(COVERED, N):
        a = np.exp(LP[:, t].astype(np.float64)) * (T64 @ a)
        c = a.sum()
        total += np.log(c)
        a /= c

    return np.float32(total)


# revision 4
# speedup vs baseline: 2.6582x; 1.3033x over previous
"""HMM log-likelihood (log-domain forward algorithm) on 8 Trainium2 cores.

Strategy: scaled linear-domain forward algorithm with warmup-halo sequence
parallelism, step-major host layout.  N=1e6 timesteps split into 8*CC
independent chains (CC/core); each chain starts from a uniform state W steps
before its owned region of L steps.  The host rearranges the per-core input
slice into X[s, partition, chain] (step-major), so the device DMA is large
contiguous packets that arrive in exactly the order the scan consumes them:
the scan overlaps the HBM stream instead of waiting behind it.

Per core, chains are batched 4-wide across the 128 SBUF partitions
(block-diagonal T^T weights on the PE) with the chain index in the matmul
free dimension; G=2 interleaved groups hide the matmul->multiply latency.
Each timestep is one bf16 matmul per group (T @ S into PSUM) plus one vector
multiply by the emission probabilities, split in half across the Vector and
GpSimd engines.  Exp (with folded bias) runs on the Scalar engine, windowed
behind the DMA.

Normalization is free: a constant per-step drift delta = E[log c] is folded
into the exp bias, making log|S| a zero-drift random walk, so no per-chain
rescaling is needed.  The bf16 quantization of T factors exactly as
D_r @ T_hat with T_hat row-stochastic; -log(r) is folded into the same exp
bias.  Each chain's contribution is log(sum(S_final)) - log(sum(S_at_W)) +
delta*L, assembled on the host, which also runs exact f64 scans for the
prefix [0, W) and the short tail.
"""

import sys

for p in ("/opt/trn_rl_repo", "/root/.axon_site", "/root/.axon_site/_ro/trn_rl_repo",
          "/root/.axon_site/_ro/pypackages"):
    if p not in sys.path:
        sys.path.insert(0, p)

import numpy as np

K = 32
N = 1_000_000
NCORES = 8
W = 8             # warmup (halo) steps per chain
L = 40            # owned steps per chain
CC = 3120         # chains per core
SPAN = W + L      # 48 sequential steps
CQ = CC // 4      # 780 chains per partition group
G = 2             # interleaved compute groups
F = CQ // G       # 390 chains (matmul free dim) per group
FH = F // 2       # 195-wide halves for the DVE/GpSimd multiply split
SB = 2            # timesteps per DMA/exp window
NW = SPAN // SB   # 24 windows
COVERED = W + NCORES * CC * L

_cache = {}


def _build():
    import concourse.bass as bass
    import concourse.bacc as bacc
    import concourse.mybir as mybir
    import concourse.tile as tile
    from contextlib import ExitStack

    f32 = mybir.dt.float32
    bf16 = mybir.dt.bfloat16
    AF = mybir.ActivationFunctionType

    nc = bacc.Bacc("TRN2", target_bir_lowering=False, debug=False,
                   num_devices=NCORES)
    # step-major input: X[s, p, j] with p = 32*q + k, chain = q*CQ + j
    x = nc.dram_tensor("x", [SPAN * 128, CQ], bf16, kind="ExternalInput")
    wmat = nc.dram_tensor("wmat", [128, 128], bf16, kind="ExternalInput")
    ebias = nc.dram_tensor("ebias", [128, 1], f32, kind="ExternalInput")
    snap_out = nc.dram_tensor("snap_out", [128, CQ], bf16, kind="ExternalOutput")
    fin_out = nc.dram_tensor("fin_out", [128, CQ], bf16, kind="ExternalOutput")

    with tile.TileContext(nc) as tc:
        with ExitStack() as ctx:
            cpool = ctx.enter_context(tc.tile_pool(name="const", bufs=1))
            rpool = ctx.enter_context(tc.tile_pool(name="rp", bufs=1))
            spool = ctx.enter_context(tc.tile_pool(name="sp", bufs=2))
            pspool = ctx.enter_context(
                tc.tile_pool(name="ps", bufs=2, space=bass.MemorySpace.PSUM))

            w_t = cpool.tile([128, 128], bf16, tag="w")
            nc.sync.dma_start(w_t[:], wmat[:])
            eb_t = cpool.tile([128, 1], f32, tag="eb")
            nc.sync.dma_start(eb_t[:], ebias[:])

            # all SPAN steps of emission data, step-major, exp'd in place
            rt = rpool.tile([128, SPAN, CQ], bf16, tag="R")

            S, SN = [], []
            for g in range(G):
                st = spool.tile([128, F], bf16, tag=f"S{g}", name=f"st{g}")
                nc.vector.memset(st[:], 1.0)
                sn = cpool.tile([128, F], bf16, tag=f"N{g}")
                S.append(st)
                SN.append(sn)

            # window DMAs (4 per window, split by partition quarter) + exp
            for w in range(NW):
                s0 = w * SB
                for q in range(4):
                    src = bass.AP(x, (s0 * 128 + 32 * q) * CQ,
                                  [[CQ, 32], [128 * CQ, SB], [1, CQ]])
                    eng = nc.sync if (w * 4 + q) % 2 == 0 else nc.gpsimd
                    eng.dma_start(rt[32 * q:32 * q + 32, s0:s0 + SB, :], src)
                nc.scalar.activation(rt[:, s0:s0 + SB, :],
                                     rt[:, s0:s0 + SB, :], AF.Exp,
                                     bias=eb_t[:])

            for s in range(SPAN):
                for g in range(G):
                    ps = pspool.tile([128, F], f32, tag=f"mm{g}")
                    nc.tensor.matmul(ps[:], w_t[:], S[g][:], start=True,
                                     stop=True)
                    sn_new = spool.tile([128, F], bf16, tag=f"S{g}",
                                        name=f"st{g}_{s}")
                    c0 = g * F
                    nc.vector.tensor_mul(sn_new[:], ps[:],
                                         rt[:, s, c0:c0 + F])
                    S[g] = sn_new
                    if s == W - 1:
                        nc.gpsimd.tensor_copy(SN[g][:], S[g][:])

            for g in range(G):
                nc.sync.dma_start(snap_out[:, g * F:(g + 1) * F], SN[g][:])
                nc.sync.dma_start(fin_out[:, g * F:(g + 1) * F], S[g][:])

    nc.compile()
    return nc


def _get_nc():
    if "nc" not in _cache:
        _cache["nc"] = _build()
    return _cache["nc"]


def _log_softmax64(v, axis):
    v = v.astype(np.float64)
    m = v.max(axis=axis, keepdims=True)
    e = np.exp(v - m)
    return v - m - np.log(e.sum(axis=axis, keepdims=True))


def _estimate_delta(log_pdf, T64):
    # E[log c] from a vectorized short scan: 64 parallel probes, 56 steps,
    # burn-in 16 (mixing time is ~10 steps).
    NCH, NST, BURN = 64, 56, 16
    cols = np.arange(NCH) * 997 + 1
    a = np.full((K, NCH), 1.0 / K)
    samples = []
    for s in range(NST):
        p = np.exp(log_pdf[:, cols + s].astype(np.float64))
        a = p * (T64 @ a)
        c = a.sum(axis=0)
        a /= c
        if s >= BURN:
            samples.append(np.log(c))
    return float(np.mean(samples))


def _make_in_maps(log_pdf, T64):
    from ml_dtypes import bfloat16

    T32 = T64.astype(np.float32)
    Tbf = T32.astype(bfloat16)
    delta = _estimate_delta(log_pdf, T64)
    # bf16-quantized T is exactly D_r @ T_hat with T_hat row-stochastic and
    # r the bf16 row sums; fold -log(r) and the drift -delta into the exp.
    r = Tbf.astype(np.float64).sum(axis=1)
    eb = np.zeros((128, 1), dtype=np.float32)
    for q in range(4):
        eb[32 * q:32 * q + 32, 0] = (-np.log(r) - delta).astype(np.float32)
    wm = np.zeros((128, 128), dtype=bfloat16)
    for q in range(4):
        wm[32 * q:32 * q + 32, 32 * q:32 * q + 32] = Tbf.T

    # step-major gather: X[s, 32q+k, j] = log_pdf[k, m*CC*L + (q*CQ+j)*L + s]
    col = np.arange(CC, dtype=np.int64) * L          # [CC]
    step = np.arange(SPAN, dtype=np.int64)           # [SPAN]
    idx0 = col[None, :] + step[:, None]              # [SPAN, CC]
    in_maps = []
    for m in range(NCORES):
        idx = m * CC * L + idx0
        xm = log_pdf[:, idx]                         # [K, SPAN, CC]
        # -> [SPAN, 4, K, CQ] -> [SPAN*128, CQ]
        xm = xm.reshape(K, SPAN, 4, CQ).transpose(1, 2, 0, 3)
        xm = np.ascontiguousarray(xm.astype(bfloat16)).reshape(SPAN * 128, CQ)
        in_maps.append({"x": xm, "wmat": wm, "ebias": eb})

    return in_maps, delta


def kernel(log_pdf: np.ndarray, pi: np.ndarray, T: np.ndarray) -> np.ndarray:
    from concourse.bass_utils import run_bass_kernel_spmd

    log_pdf = np.ascontiguousarray(log_pdf, dtype=np.float32)
    log_pi64 = _log_softmax64(pi, 0)
    log_T64 = _log_softmax64(T, 1)
    T64 = np.exp(log_T64)                     # row-stochastic [K, K] f64

    in_maps, delta = _make_in_maps(log_pdf, T64)
    nc = _get_nc()
    res = run_bass_kernel_spmd(nc, in_maps, list(range(NCORES))).results

    # ---- host combine (f64) ----
    LP = log_pdf
    # exact prefix [0, W)
    a = np.exp(log_pi64 + LP[:, 0].astype(np.float64))
    c = a.sum()
    total = np.log(c)
    a /= c
    for t in range(1, W):
        a = np.exp(LP[:, t].astype(np.float64)) * (T64 @ a)
        c = a.sum()
        total += np.log(c)
        a /= c

    # per-chain contributions: log(sum fin) - log(sum snap) + delta*L
    for m in range(NCORES):
        snap = res[m]["snap_out"].astype(np.float64)   # [128, CQ]
        fin = res[m]["fin_out"].astype(np.float64)
        for q in range(4):
            ssum = snap[32 * q:32 * q + 32, :].sum(axis=0)
            fsum = fin[32 * q:32 * q + 32, :].sum(axis=0)
            total += (np.log(fsum) - np.log(ssum)).sum() + delta * L * CQ

    # exact tail [COVERED, N) from the last chain's final state
    # last chain = (core NCORES-1, q=3, j=CQ-1)
    fv = res[NCORES - 1]["fin_out"][96:128, CQ - 1].astype(np.float64)
    a = fv / fv.sum()
    for t in range(COVERED, N):
        a = np.exp(LP[:, t].astype(np.float64)) * (T64 @ a)
        c = a.sum()
        total += np.log(c)
        a /= c

    return np.float32(total)


# revision 5
# speedup vs baseline: 2.9901x; 1.1249x over previous
"""HMM log-likelihood (log-domain forward algorithm) on 8 Trainium2 cores.

Strategy: scaled linear-domain forward algorithm with warmup-halo sequence
parallelism, step-major host layout.  N=1e6 timesteps split into 8*CC
independent chains (CC/core); each chain starts from a uniform state W steps
before its owned region of L steps.  The host rearranges the per-core input
slice into X[s, partition, chain] (step-major), so the device DMA is large
contiguous packets that arrive in exactly the order the scan consumes them:
the scan overlaps the HBM stream instead of waiting behind it.

Per core, chains are batched 4-wide across the 128 SBUF partitions
(block-diagonal T^T weights on the PE) with the chain index in the matmul
free dimension; G=2 interleaved groups hide the matmul->multiply latency.
Each timestep is one bf16 matmul per group (T @ S into PSUM) plus one vector
multiply by the emission probabilities, split in half across the Vector and
GpSimd engines.  Exp (with folded bias) runs on the Scalar engine, windowed
behind the DMA.

Normalization is free: a constant per-step drift delta = E[log c] is folded
into the exp bias, making log|S| a zero-drift random walk, so no per-chain
rescaling is needed.  The bf16 quantization of T factors exactly as
D_r @ T_hat with T_hat row-stochastic; -log(r) is folded into the same exp
bias.  Each chain's contribution is log(sum(S_final)) - log(sum(S_at_W)) +
delta*L, assembled on the host, which also runs exact f64 scans for the
prefix [0, W) and the short tail.
"""

import sys

for p in ("/opt/trn_rl_repo", "/root/.axon_site", "/root/.axon_site/_ro/trn_rl_repo",
          "/root/.axon_site/_ro/pypackages"):
    if p not in sys.path:
        sys.path.insert(0, p)

import numpy as np

K = 32
N = 1_000_000
NCORES = 8
W = 8             # warmup (halo) steps per chain
L = 40            # owned steps per chain
CC = 3120         # chains per core
SPAN = W + L      # 48 sequential steps
CQ = CC // 4      # 780 chains per partition group
G = 2             # interleaved compute groups
F = CQ // G       # 390 chains (matmul free dim) per group
FH = F // 2       # 195-wide halves for the DVE/GpSimd multiply split
WINS = [1, 1] + [2] * 23   # per-window step counts (sum = SPAN)
assert sum(WINS) == SPAN
COVERED = W + NCORES * CC * L

_cache = {}


def _build():
    import concourse.bass as bass
    import concourse.bacc as bacc
    import concourse.mybir as mybir
    import concourse.tile as tile
    from contextlib import ExitStack

    f32 = mybir.dt.float32
    bf16 = mybir.dt.bfloat16
    fp8 = mybir.dt.float8e4
    AF = mybir.ActivationFunctionType

    nc = bacc.Bacc("TRN2", target_bir_lowering=False, debug=False,
                   num_devices=NCORES)
    # step-major input: X[s, p, j] with p = 32*q + k, chain = q*CQ + j
    x = nc.dram_tensor("x", [SPAN * 128, CQ], bf16, kind="ExternalInput")
    wmat = nc.dram_tensor("wmat", [128, 128], fp8, kind="ExternalInput")
    ebias = nc.dram_tensor("ebias", [128, 1], f32, kind="ExternalInput")
    snap_out = nc.dram_tensor("snap_out", [128, CQ], bf16, kind="ExternalOutput")
    fin_out = nc.dram_tensor("fin_out", [128, CQ], bf16, kind="ExternalOutput")

    with tile.TileContext(nc) as tc:
        with ExitStack() as ctx:
            cpool = ctx.enter_context(tc.tile_pool(name="const", bufs=1))
            rpool = ctx.enter_context(tc.tile_pool(name="rp", bufs=1))
            xpool = ctx.enter_context(tc.tile_pool(name="xp", bufs=4))
            spool = ctx.enter_context(tc.tile_pool(name="sp", bufs=2))
            pspool = ctx.enter_context(
                tc.tile_pool(name="ps", bufs=2, space=bass.MemorySpace.PSUM))

            w_t = cpool.tile([128, 128], fp8, tag="w")
            nc.sync.dma_start(w_t[:], wmat[:])
            eb_t = cpool.tile([128, 1], f32, tag="eb")
            nc.sync.dma_start(eb_t[:], ebias[:])

            # all SPAN steps of emission data, step-major, exp'd in place
            rt = rpool.tile([128, SPAN, CQ], f32, tag="R")

            S, SN = [], []
            for g in range(G):
                st = spool.tile([128, F], bf16, tag=f"S{g}", name=f"st{g}")
                nc.vector.memset(st[:], 1.0)
                sn = cpool.tile([128, F], bf16, tag=f"N{g}")
                S.append(st)
                SN.append(sn)

            # window DMAs (4 per window, split by partition quarter) into a
            # bf16 staging tile, then exp -> the f32 R tile
            s0 = 0
            for w, sb in enumerate(WINS):
                xt = xpool.tile([128, sb, CQ], bf16, tag=f"X{sb}",
                                name=f"xt{w}")
                for q in range(4):
                    src = bass.AP(x, (s0 * 128 + 32 * q) * CQ,
                                  [[CQ, 32], [128 * CQ, sb], [1, CQ]])
                    eng = nc.sync if (w * 4 + q) % 2 == 0 else nc.gpsimd
                    eng.dma_start(xt[32 * q:32 * q + 32, :, :], src)
                nc.scalar.activation(rt[:, s0:s0 + sb, :], xt[:],
                                     AF.Exp, bias=eb_t[:])
                s0 += sb

            for s in range(SPAN):
                for g in range(G):
                    ps = pspool.tile([128, F], f32, tag=f"mm{g}")
                    nc.tensor.matmul(ps[:], w_t[:], S[g][:], start=True,
                                     stop=True)
                    sn_new = spool.tile([128, F], bf16, tag=f"S{g}",
                                        name=f"st{g}_{s}")
                    c0 = g * F
                    nc.vector.tensor_mul(sn_new[:], ps[:],
                                         rt[:, s, c0:c0 + F])
                    S[g] = sn_new
                    if s == W - 1:
                        nc.gpsimd.tensor_copy(SN[g][:], S[g][:])
                        nc.sync.dma_start(
                            snap_out[:, g * F:(g + 1) * F], SN[g][:])

            for g in range(G):
                h = F // 2
                nc.sync.dma_start(
                    fin_out[:, g * F:g * F + h], S[g][:, 0:h])
                nc.gpsimd.dma_start(
                    fin_out[:, g * F + h:(g + 1) * F], S[g][:, h:F])

    nc.compile()
    return nc


def _get_nc():
    if "nc" not in _cache:
        _cache["nc"] = _build()
    return _cache["nc"]


def _log_softmax64(v, axis):
    v = v.astype(np.float64)
    m = v.max(axis=axis, keepdims=True)
    e = np.exp(v - m)
    return v - m - np.log(e.sum(axis=axis, keepdims=True))


def _estimate_delta(log_pdf, T64):
    # E[log c] from a vectorized short scan: 64 parallel probes, 56 steps,
    # burn-in 16 (mixing time is ~10 steps).
    NCH, NST, BURN = 64, 56, 16
    cols = np.arange(NCH) * 997 + 1
    a = np.full((K, NCH), 1.0 / K)
    samples = []
    for s in range(NST):
        p = np.exp(log_pdf[:, cols + s].astype(np.float64))
        a = p * (T64 @ a)
        c = a.sum(axis=0)
        a /= c
        if s >= BURN:
            samples.append(np.log(c))
    return float(np.mean(samples))


def _make_in_maps(log_pdf, T64):
    from ml_dtypes import bfloat16, float8_e4m3fn

    T32 = T64.astype(np.float32)
    Tq = T32.astype(float8_e4m3fn)
    delta = _estimate_delta(log_pdf, T64)
    # fp8-quantized T is exactly D_r @ T_hat with T_hat row-stochastic and
    # r the fp8 row sums; fold -log(r) and the drift -delta into the exp.
    r = Tq.astype(np.float64).sum(axis=1)
    eb = np.zeros((128, 1), dtype=np.float32)
    for q in range(4):
        eb[32 * q:32 * q + 32, 0] = (-np.log(r) - delta).astype(np.float32)
    wm = np.zeros((128, 128), dtype=float8_e4m3fn)
    for q in range(4):
        wm[32 * q:32 * q + 32, 32 * q:32 * q + 32] = Tq.T

    # step-major gather: X[s, 32q+k, j] = log_pdf[k, m*CC*L + (q*CQ+j)*L + s]
    col = np.arange(CC, dtype=np.int64) * L          # [CC]
    step = np.arange(SPAN, dtype=np.int64)           # [SPAN]
    idx0 = col[None, :] + step[:, None]              # [SPAN, CC]
    in_maps = []
    for m in range(NCORES):
        idx = m * CC * L + idx0
        xm = log_pdf[:, idx]                         # [K, SPAN, CC]
        # -> [SPAN, 4, K, CQ] -> [SPAN*128, CQ]
        xm = xm.reshape(K, SPAN, 4, CQ).transpose(1, 2, 0, 3)
        xm = np.ascontiguousarray(xm.astype(bfloat16)).reshape(SPAN * 128, CQ)
        in_maps.append({"x": xm, "wmat": wm, "ebias": eb})

    return in_maps, delta


def kernel(log_pdf: np.ndarray, pi: np.ndarray, T: np.ndarray) -> np.ndarray:
    from concourse.bass_utils import run_bass_kernel_spmd

    log_pdf = np.ascontiguousarray(log_pdf, dtype=np.float32)
    log_pi64 = _log_softmax64(pi, 0)
    log_T64 = _log_softmax64(T, 1)
    T64 = np.exp(log_T64)                     # row-stochastic [K, K] f64

    in_maps, delta = _make_in_maps(log_pdf, T64)
    nc = _get_nc()
    res = run_bass_kernel_spmd(nc, in_maps, list(range(NCORES))).results

    # ---- host combine (f64) ----
    LP = log_pdf
    # exact prefix [0, W)
    a = np.exp(log_pi64 + LP[:, 0].astype(np.float64))
    c = a.sum()
    total = np.log(c)
    a /= c
    for t in range(1, W):
        a = np.exp(LP[:, t].astype(np.float64)) * (T64 @ a)
        c = a.sum()
        total += np.log(c)
        a /= c

    # per-chain contributions: log(sum fin) - log(sum snap) + delta*L
    for m in range(NCORES):
        snap = res[m]["snap_out"].astype(np.float64)   # [128, CQ]
        fin = res[m]["fin_out"].astype(np.float64)
        for q in range(4):
            ssum = snap[32 * q:32 * q + 32, :].sum(axis=0)
            fsum = fin[32 * q:32 * q + 32, :].sum(axis=0)
            total += (np.log(fsum) - np.log(ssum)).sum() + delta * L * CQ

    # exact tail [COVERED, N) from the last chain's final state
    # last chain = (core NCORES-1, q=3, j=CQ-1)
    fv = res[NCORES - 1]["fin_out"][96:128, CQ - 1].astype(np.float64)
    a = fv / fv.sum()
    for t in range(COVERED, N):
        a = np.exp(LP[:, t].astype(np.float64)) * (T64 @ a)
        c = a.sum()
        total += np.log(c)
        a /= c

    return np.float32(total)


# revision 8
# speedup vs baseline: 3.2075x; 1.0727x over previous
"""HMM log-likelihood (log-domain forward algorithm) on 8 Trainium2 cores.

Strategy: scaled linear-domain forward algorithm with warmup-halo sequence
parallelism, step-major host layout.  N=1e6 timesteps split into 8*CC
independent chains (CC/core); each chain starts from a uniform state W steps
before its owned region of L steps.  The host rearranges the per-core input
slice into X[s, partition, chain] (step-major), so the device DMA is large
contiguous packets that arrive in exactly the order the scan consumes them:
the scan overlaps the HBM stream instead of waiting behind it.

Per core, chains are batched 4-wide across the 128 SBUF partitions
(block-diagonal T^T weights on the PE) with the chain index in the matmul
free dimension; G=2 interleaved groups hide the matmul->multiply latency.
Each timestep is one bf16 matmul per group (T @ S into PSUM) plus one vector
multiply by the emission probabilities, split in half across the Vector and
GpSimd engines.  Exp (with folded bias) runs on the Scalar engine, windowed
behind the DMA.

Normalization is free: a constant per-step drift delta = E[log c] is folded
into the exp bias, making log|S| a zero-drift random walk, so no per-chain
rescaling is needed.  The bf16 quantization of T factors exactly as
D_r @ T_hat with T_hat row-stochastic; -log(r) is folded into the same exp
bias.  Each chain's contribution is log(sum(S_final)) - log(sum(S_at_W)) +
delta*L, assembled on the host, which also runs exact f64 scans for the
prefix [0, W) and the short tail.
"""

import sys

for p in ("/opt/trn_rl_repo", "/root/.axon_site", "/root/.axon_site/_ro/trn_rl_repo",
          "/root/.axon_site/_ro/pypackages"):
    if p not in sys.path:
        sys.path.insert(0, p)

import numpy as np

K = 32
N = 1_000_000
NCORES = 8
W = 6             # warmup (halo) steps per chain
L = 32            # owned steps per chain
CC = 3904         # chains per core
SPAN = W + L      # 38 sequential steps
CQ = CC // 4      # 976 chains per partition group
G = 2             # interleaved compute groups
F = CQ // G       # 488 chains (matmul free dim) per group
WINS = [1, 1] + [2] * 18   # per-window step counts (sum = SPAN)
assert sum(WINS) == SPAN
COVERED = W + NCORES * CC * L

_cache = {}


def _build():
    import concourse.bass as bass
    import concourse.bacc as bacc
    import concourse.mybir as mybir
    import concourse.tile as tile
    from contextlib import ExitStack

    f32 = mybir.dt.float32
    bf16 = mybir.dt.bfloat16
    fp8 = mybir.dt.float8e4
    AF = mybir.ActivationFunctionType

    nc = bacc.Bacc("TRN2", target_bir_lowering=False, debug=False,
                   num_devices=NCORES)
    # step-major input: X[s, p, j] with p = 32*q + k, chain = q*CQ + j
    x = nc.dram_tensor("x", [SPAN * 128, CQ], bf16, kind="ExternalInput")
    wmat = nc.dram_tensor("wmat", [128, 128], fp8, kind="ExternalInput")
    ebias = nc.dram_tensor("ebias", [128, 1], f32, kind="ExternalInput")
    snap_out = nc.dram_tensor("snap_out", [4, CQ], f32, kind="ExternalOutput")
    fin_out = nc.dram_tensor("fin_out", [4, CQ], f32, kind="ExternalOutput")

    with tile.TileContext(nc) as tc:
        with ExitStack() as ctx:
            cpool = ctx.enter_context(tc.tile_pool(name="const", bufs=1))
            rpool = ctx.enter_context(tc.tile_pool(name="rp", bufs=1))
            xpool = ctx.enter_context(tc.tile_pool(name="xp", bufs=4))
            spool = ctx.enter_context(tc.tile_pool(name="sp", bufs=2))
            pspool = ctx.enter_context(
                tc.tile_pool(name="ps", bufs=2, space=bass.MemorySpace.PSUM))

            w_t = cpool.tile([128, 128], fp8, tag="w")
            nc.sync.dma_start(w_t[:], wmat[:])
            ones_t = cpool.tile([128, 4], fp8, tag="ones")
            nc.vector.memset(ones_t[:], 0.0)
            for q in range(4):
                nc.vector.memset(ones_t[32 * q:32 * q + 32, q:q + 1], 1.0)
            eb_t = cpool.tile([128, 1], f32, tag="eb")
            nc.sync.dma_start(eb_t[:], ebias[:])

            # all SPAN steps of emission data, step-major, exp'd in place
            rt = rpool.tile([128, SPAN, CQ], f32, tag="R")

            S, SN = [], []
            for g in range(G):
                st = spool.tile([128, F], bf16, tag=f"S{g}", name=f"st{g}")
                nc.vector.memset(st[:], 1.0)
                sn = cpool.tile([128, F], bf16, tag=f"N{g}")
                S.append(st)
                SN.append(sn)

            # window DMAs (4 per window, split by partition quarter) into a
            # bf16 staging tile, then exp -> the f32 R tile
            s0 = 0
            for w, sb in enumerate(WINS):
                xt = xpool.tile([128, sb, CQ], bf16, tag=f"X{sb}",
                                name=f"xt{w}")
                for q in range(4):
                    src = bass.AP(x, (s0 * 128 + 32 * q) * CQ,
                                  [[CQ, 32], [128 * CQ, sb], [1, CQ]])
                    if w < 2:
                        eng = [nc.sync, nc.gpsimd, nc.scalar, nc.sync][q]
                    else:
                        eng = nc.sync if (w * 4 + q) % 2 == 0 else nc.gpsimd
                    eng.dma_start(xt[32 * q:32 * q + 32, :, :], src)
                nc.scalar.activation(rt[:, s0:s0 + sb, :], xt[:],
                                     AF.Exp, bias=eb_t[:])
                s0 += sb

            for s in range(SPAN):
                for g in range(G):
                    ps = pspool.tile([128, F], f32, tag=f"mm{g}")
                    nc.tensor.matmul(ps[:], w_t[:], S[g][:], start=True,
                                     stop=True)
                    sn_new = spool.tile([128, F], bf16, tag=f"S{g}",
                                        name=f"st{g}_{s}")
                    c0 = g * F
                    nc.vector.tensor_mul(sn_new[:], ps[:],
                                         rt[:, s, c0:c0 + F])
                    S[g] = sn_new
                    if s == W - 1:
                        nc.gpsimd.tensor_copy(SN[g][:], S[g][:])
                if s == W:
                    # off-chain: 32->1 partition sums of the snapshot
                    for g in range(G):
                        pss = pspool.tile([4, F], f32, tag=f"sn{g}")
                        nc.tensor.matmul(pss[:], ones_t[:], SN[g][:],
                                         start=True, stop=True)
                        sns = cpool.tile([4, F], f32, tag=f"sns{g}")
                        nc.scalar.copy(sns[:], pss[:])
                        nc.sync.dma_start(
                            snap_out[:, g * F:(g + 1) * F], sns[:])

            for g in range(G):
                psf = pspool.tile([4, F], f32, tag=f"sn{g}")
                nc.tensor.matmul(psf[:], ones_t[:], S[g][:],
                                 start=True, stop=True)
                fns = cpool.tile([4, F], f32, tag=f"fns{g}")
                nc.scalar.copy(fns[:], psf[:])
                eng = nc.sync if g == 0 else nc.gpsimd
                eng.dma_start(fin_out[:, g * F:(g + 1) * F], fns[:])

    nc.compile()
    return nc


def _get_nc():
    if "nc" not in _cache:
        _cache["nc"] = _build()
    return _cache["nc"]


def _log_softmax64(v, axis):
    v = v.astype(np.float64)
    m = v.max(axis=axis, keepdims=True)
    e = np.exp(v - m)
    return v - m - np.log(e.sum(axis=axis, keepdims=True))


def _estimate_delta(log_pdf, T64):
    # E[log c] from a vectorized short scan: 64 parallel probes, 56 steps,
    # burn-in 16 (mixing time is ~10 steps).
    NCH, NST, BURN = 64, 56, 16
    cols = np.arange(NCH) * 997 + 1
    a = np.full((K, NCH), 1.0 / K)
    samples = []
    for s in range(NST):
        p = np.exp(log_pdf[:, cols + s].astype(np.float64))
        a = p * (T64 @ a)
        c = a.sum(axis=0)
        a /= c
        if s >= BURN:
            samples.append(np.log(c))
    return float(np.mean(samples))


def _make_in_maps(log_pdf, T64):
    from ml_dtypes import bfloat16, float8_e4m3fn

    T32 = T64.astype(np.float32)
    Tq = T32.astype(float8_e4m3fn)
    delta = _estimate_delta(log_pdf, T64)
    # fp8-quantized T is exactly D_r @ T_hat with T_hat row-stochastic and
    # r the fp8 row sums; fold -log(r) and the drift -delta into the exp.
    r = Tq.astype(np.float64).sum(axis=1)
    eb = np.zeros((128, 1), dtype=np.float32)
    for q in range(4):
        eb[32 * q:32 * q + 32, 0] = (-np.log(r) - delta).astype(np.float32)
    wm = np.zeros((128, 128), dtype=float8_e4m3fn)
    for q in range(4):
        wm[32 * q:32 * q + 32, 32 * q:32 * q + 32] = Tq.T

    # step-major gather: X[s, 32q+k, j] = log_pdf[k, m*CC*L + (q*CQ+j)*L + s]
    col = np.arange(CC, dtype=np.int64) * L          # [CC]
    step = np.arange(SPAN, dtype=np.int64)           # [SPAN]
    idx0 = col[None, :] + step[:, None]              # [SPAN, CC]
    in_maps = []
    for m in range(NCORES):
        idx = m * CC * L + idx0
        xm = log_pdf[:, idx]                         # [K, SPAN, CC]
        # -> [SPAN, 4, K, CQ] -> [SPAN*128, CQ]
        xm = xm.reshape(K, SPAN, 4, CQ).transpose(1, 2, 0, 3)
        xm = np.ascontiguousarray(xm.astype(bfloat16)).reshape(SPAN * 128, CQ)
        in_maps.append({"x": xm, "wmat": wm, "ebias": eb})

    return in_maps, delta


def kernel(log_pdf: np.ndarray, pi: np.ndarray, T: np.ndarray) -> np.ndarray:
    from concourse.bass_utils import run_bass_kernel_spmd

    log_pdf = np.ascontiguousarray(log_pdf, dtype=np.float32)
    log_pi64 = _log_softmax64(pi, 0)
    log_T64 = _log_softmax64(T, 1)
    T64 = np.exp(log_T64)                     # row-stochastic [K, K] f64

    in_maps, delta = _make_in_maps(log_pdf, T64)
    nc = _get_nc()
    res = run_bass_kernel_spmd(nc, in_maps, list(range(NCORES))).results

    # ---- host combine (f64) ----
    LP = log_pdf
    # exact prefix [0, W)
    a = np.exp(log_pi64 + LP[:, 0].astype(np.float64))
    c = a.sum()
    total = np.log(c)
    a /= c
    for t in range(1, W):
        a = np.exp(LP[:, t].astype(np.float64)) * (T64 @ a)
        c = a.sum()
        total += np.log(c)
        a /= c

    # per-chain contributions: log(sum fin) - log(sum snap) + delta*L
    for m in range(NCORES):
        ssum = res[m]["snap_out"].astype(np.float64)   # [4, CQ]
        fsum = res[m]["fin_out"].astype(np.float64)
        total += (np.log(fsum) - np.log(ssum)).sum() + delta * L * CQ * 4

    # exact tail [COVERED, N) from the last covered column's true filter:
    # recompute it exactly on the host over the last chain's span
    mlast = NCORES - 1
    c_last = mlast * CC * L + (CC - 1) * L
    a = np.full(K, 1.0 / K)
    for t in range(c_last, COVERED):
        a = np.exp(LP[:, t].astype(np.float64)) * (T64 @ a)
        a /= a.sum()
    for t in range(COVERED, N):
        a = np.exp(LP[:, t].astype(np.float64)) * (T64 @ a)
        c = a.sum()
        total += np.log(c)
        a /= c

    return np.float32(total)
